# revision 1
# baseline (speedup 1.0000x reference)
"""ConcatCritic pair-MLP kernel for 8 Trainium2 NeuronCores.

scores[i, j] = MLP(concat(x_i, y_j)) with
MLP = Linear(256,512) -> ReLU -> Linear(512,512) -> ReLU -> Linear(512,1).

Sharding: pure data parallelism over the B^2 pair grid, split along the
x (row) index — each core gets 64 rows of x plus all of y and the full
(small) weight set, and produces a [64, 512] slab of the output.

The host passes x and y pre-transposed (xT [128,64] slab, yT [128,512]) and
receives the output in transposed chunk layout [4][128 j][64 i]; both
transposes are trivial numpy work and remove every on-device transpose.

Per-core dataflow (per x-row i):
  stage A (ACT): h1T[h, j] = relu(hyT[h, j] + (hx_i[h] + b1[h]))     4x [128,512]
  stage B (PE):  x2[j, k]  = h1_i @ W2   (fp32r matmuls, psum accum) 16x mm
  stage C (DVE): s_i[j]    = sum_k relu(x2[j, k]) * W3[k]            4x scalar_tensor_tensor
with hxT = (x @ W1[:128]).T + b1 and hyT = (y @ W1[128:]).T computed once at
setup. b3 is applied on the host; b2 (nonzero only) via an extra K=1 matmul.

Measured on the 8-core axon TRN2: ~262us HW exec (NTFF-profiled), ~92% of
the fp32r PE arithmetic floor (218.5us of 128x128-MAC column streaming per
core), with the PE issuing back-to-back matmuls at ~227-235ns per N=512
group and zero >1us stalls; the remainder is ~16us input-DMA/setup ramp and
~11us Tile drain barrier. Accuracy vs the fp32 reference: 2.5e-4 absmax-rel.
"""

import numpy as np

B = 512
DX = 128
H = 512
N_CORES = 8
ROWS = B // N_CORES  # 64 x-rows per core
HC = H // 128  # 4 chunks of the hidden dim

_BUILT = {}  # (with_b2, use_f32r) -> bass.Bass


def _build(with_b2: bool, use_f32r: bool = True):
    import concourse.mybir as mybir
    from concourse.bacc import Bacc
    from concourse.tile import TileContext

    F32 = mybir.dt.float32
    # fp32r = fp32 rounded to the PE's fast-path input precision: 1 cycle/row
    # instead of 4 at N>=256. The BIR verifier requires every producer of an
    # fp32r matmul operand to declare its output float32r, so operand tiles
    # carry this dtype and the input DMAs bitcast the dram side to match.
    MMDT = mybir.dt.float32r if use_f32r else mybir.dt.float32
    Relu = mybir.ActivationFunctionType.Relu
    Alu = mybir.AluOpType

    # Bacc (not raw Bass): its compile pipeline splits multi-semaphore waits
    # into event-semaphore chains — TRN2 engine instructions accept only one
    # sync wait, which walrus otherwise rejects ("Too many sync wait
    # commands").
    nc = Bacc()
    xT_d = nc.declare_dram_parameter("xT", [DX, ROWS], F32, isOutput=False)
    yT_d = nc.declare_dram_parameter("yT", [DX, B], F32, isOutput=False)
    w1_d = nc.declare_dram_parameter("W1", [2 * DX, H], F32, isOutput=False)
    b1_d = nc.declare_dram_parameter("b1", [H], F32, isOutput=False)
    w2_d = nc.declare_dram_parameter("W2", [H, H], F32, isOutput=False)
    w3_d = nc.declare_dram_parameter("W3", [H, 1], F32, isOutput=False)
    if with_b2:
        b2_d = nc.declare_dram_parameter("b2", [H], F32, isOutput=False)
    out_d = nc.declare_dram_parameter("outT", [HC, 128, ROWS], F32, isOutput=True)

    with TileContext(nc) as tc:
        with (
            tc.tile_pool(name="consts", bufs=1) as cpool,
            tc.tile_pool(name="work", bufs=2) as wpool,
            tc.tile_pool(name="psum", bufs=8, space="PSUM") as ppool,
        ):
            # ---------------- input DMAs ----------------
            # Matmul operands are DMA'd straight into float32r tiles via a
            # dram-side bitcast (same 4 bytes; the PE rounds its inputs on
            # read, so a producer-side round-copy adds nothing but latency).
            # DMAs spread over the three DGE rings (sync/SP, scalar, gpsimd)
            # so the y-path, x-path and W2 stream in parallel; within each
            # ring the earliest-needed tensor goes first.
            def bc(ap):
                return ap.bitcast(MMDT) if use_f32r else ap

            # sync ring: yT and the x slab (gates hy/hx), then W2 chunks.
            yT = cpool.tile([DX, B], MMDT, name="yT")
            nc.sync.dma_start(out=yT[:], in_=bc(yT_d[:, :]))
            xT = cpool.tile([DX, ROWS], MMDT, name="xT")
            nc.sync.dma_start(out=xT[:], in_=bc(xT_d[:, :]))
            w2sb = [cpool.tile([128, H], MMDT, name=f"w2_{hc}") for hc in range(HC)]
            nc.sync.dma_start(out=w2sb[0][:], in_=bc(w2_d[0:128, :]))
            nc.sync.dma_start(out=w2sb[3][:], in_=bc(w2_d[3 * 128 : 4 * 128, :]))
            # scalar ring: w1y (gates hy), then W2 chunk 1.
            w1y = cpool.tile([DX, H], MMDT, name="w1y")
            nc.scalar.dma_start(out=w1y[:], in_=bc(w1_d[DX : 2 * DX, :]))
            nc.scalar.dma_start(out=w2sb[1][:], in_=bc(w2_d[128:256, :]))
            # gpsimd ring: w1x (gates hx) first, then the small tensors and
            # W2 chunk 2.
            w1x = cpool.tile([DX, H], MMDT, name="w1x")
            nc.gpsimd.dma_start(out=w1x[:], in_=bc(w1_d[0:DX, :]))
            w3row = cpool.tile([1, H], F32, name="w3row")
            nc.gpsimd.dma_start(out=w3row[:], in_=w3_d[:, :].flatten().unsqueeze(0))
            b1sb = cpool.tile([128, HC], F32, name="b1sb")  # [p, hc]
            nc.gpsimd.dma_start(out=b1sb[:], in_=b1_d[:].rearrange("(c p) -> p c", p=128))
            nc.gpsimd.dma_start(out=w2sb[2][:], in_=bc(w2_d[2 * 128 : 3 * 128, :]))
            # W3 broadcast on-chip (a partition-stride-0 DMA would re-read
            # the 2KB row 128 times from HBM).
            w3b = cpool.tile([128, H], F32, name="w3b")
            nc.gpsimd.partition_broadcast(w3b[:], w3row[:])
            if with_b2:
                b2row = cpool.tile([1, H], MMDT, name="b2row")
                nc.sync.dma_start(out=b2row[:], in_=bc(b2_d[:].unsqueeze(0)))
                # memset can't write float32r; fill fp32 then round-copy
                ones_f = cpool.tile([1, 128], F32, name="ones_f")
                nc.vector.memset(ones_f[:], 1.0)
                ones1 = cpool.tile([1, 128], MMDT, name="ones1")
                nc.vector.tensor_copy(out=ones1[:], in_=ones_f[:])

            # hyT[hc][h, j] = (y @ W1y).T
            hyT = []
            for hc in range(HC):
                pshy = ppool.tile([128, B], F32, name="pshy", tag="ps")
                nc.tensor.matmul(
                    pshy[:], w1y[:, hc * 128 : (hc + 1) * 128], yT[:], start=True, stop=True
                )
                t = cpool.tile([128, B], F32, name=f"hyT_{hc}")
                nc.vector.tensor_copy(out=t[:], in_=pshy[:])
                hyT.append(t)

            # hxT[hc][h, i] = (x @ W1x).T + b1
            hxT = []
            for hc in range(HC):
                pshx = ppool.tile([128, ROWS], F32, name="pshx", tag="ps")
                nc.tensor.matmul(
                    pshx[:], w1x[:, hc * 128 : (hc + 1) * 128], xT[:], start=True, stop=True
                )
                t = cpool.tile([128, ROWS], F32, name=f"hxT_{hc}")
                nc.vector.tensor_scalar_add(t[:], pshx[:], b1sb[:, hc : hc + 1])
                hxT.append(t)

            # scores accumulated transposed: scoresT[jc][j, i]
            scoresT = [cpool.tile([128, ROWS], F32, name=f"scT_{jc}") for jc in range(HC)]

            # ---------------- main loop over x rows ----------------
            for i in range(ROWS):
                h1T = []
                for hc in range(HC):
                    # ACT: relu(hyT + hx_i). All of stage A lives on ACT so the
                    # DVE has headroom for the stage-C fused reduce — DVE is
                    # otherwise the bottleneck engine (measured 92% busy).
                    t = wpool.tile([128, B], MMDT, name="h1T", tag="h1T", bufs=12)
                    nc.scalar.activation(
                        t[:], hyT[hc][:], Relu, bias=hxT[hc][:, i : i + 1], scale=1.0
                    )
                    h1T.append(t)
                for jc in range(HC):
                    ps2 = ppool.tile([128, B], F32, name="ps2", tag="ps")
                    for hc in range(HC):
                        nc.tensor.matmul(
                            ps2[:],
                            h1T[hc][:, jc * 128 : (jc + 1) * 128],
                            w2sb[hc][:],
                            start=(hc == 0),
                            stop=(hc == HC - 1 and not with_b2),
                        )
                    if with_b2:
                        nc.tensor.matmul(
                            ps2[:], ones1[:], b2row[:], start=False, stop=True
                        )
                    # DVE: scr = relu(ps2) * W3_bcast; scoresT col = sum_k scr
                    scr = wpool.tile([128, B], F32, name="scr", tag="scr", bufs=6)
                    nc.vector.scalar_tensor_tensor(
                        out=scr[:],
                        in0=ps2[:],
                        scalar=0.0,
                        in1=w3b[:],
                        op0=Alu.max,
                        op1=Alu.mult,
                        accum_out=scoresT[jc][:, i : i + 1],
                    )

            # ---------------- store (host un-transposes) ----------------
            for jc in range(HC):
                nc.sync.dma_start(out=out_d[jc, :, :], in_=scoresT[jc][:])

    nc.finalize()  # runs the Bacc pass pipeline (wait splitting etc.)
    return nc


def _get_nc(with_b2: bool, use_f32r: bool = True):
    key = (with_b2, use_f32r)
    if key not in _BUILT:
        _BUILT[key] = _build(with_b2, use_f32r)
    return _BUILT[key]


def _run(inputs: dict, trace: bool = False, use_f32r: bool = True, **spmd_kwargs):
    """Shard, execute on 8 cores, gather. Returns (scores, BassKernelResults)."""
    from concourse.bass_utils import run_bass_kernel_spmd

    x = np.asarray(inputs["x"], dtype=np.float32)
    y = np.asarray(inputs["y"], dtype=np.float32)
    W1 = np.ascontiguousarray(np.asarray(inputs["W1"], dtype=np.float32))
    b1 = np.ascontiguousarray(np.asarray(inputs["b1"], dtype=np.float32))
    W2 = np.ascontiguousarray(np.asarray(inputs["W2"], dtype=np.float32))
    b2 = np.ascontiguousarray(np.asarray(inputs.get("b2", np.zeros(H)), dtype=np.float32))
    W3 = np.ascontiguousarray(np.asarray(inputs["W3"], dtype=np.float32))
    b3 = np.asarray(inputs.get("b3", np.zeros(1)), dtype=np.float32)

    with_b2 = bool(np.any(b2))
    nc = _get_nc(with_b2, use_f32r)

    yT = np.ascontiguousarray(y.T)
    in_maps = []
    for c in range(N_CORES):
        m = {
            "xT": np.ascontiguousarray(x[c * ROWS : (c + 1) * ROWS].T),
            "yT": yT,
            "W1": W1,
            "b1": b1,
            "W2": W2,
            "W3": W3,
        }
        if with_b2:
            m["b2"] = b2
        in_maps.append(m)

    res = run_bass_kernel_spmd(
        nc, in_maps, core_ids=list(range(N_CORES)), trace=trace, **spmd_kwargs
    )
    # outT[jc, j, i] -> scores_slab[i, jc*128 + j]
    slabs = [
        np.transpose(r["outT"], (2, 0, 1)).reshape(ROWS, B) for r in res.results
    ]
    out = np.concatenate(slabs, axis=0)
    if b3.size and np.any(b3):
        out = out + b3.reshape(-1)[0]
    return np.ascontiguousarray(out.astype(np.float32)), res


def kernel(**inputs) -> np.ndarray:
    out, _ = _run(inputs)
    return out



# revision 2
# speedup vs baseline: 1.0576x; 1.0576x over previous
"""ConcatCritic pair-MLP kernel for 8 Trainium2 NeuronCores.

scores[i, j] = MLP(concat(x_i, y_j)) with
MLP = Linear(256,512) -> ReLU -> Linear(512,512) -> ReLU -> Linear(512,1).

Sharding: pure data parallelism over the B^2 pair grid, split along the
x (row) index — each core gets 64 rows of x plus all of y and the full
(small) weight set, and produces a [64, 512] slab of the output.

The host passes x and y pre-transposed (xT [128,64] slab, yT [128,512]) and
receives the output in transposed chunk layout [4][128 j][64 i]; both
transposes are trivial numpy work and remove every on-device transpose.

All matmul operands are fp16 (host-converted): the PE runs fp16 at the same
1 cycle/row as fp32r, but fp16 stationaries re-enable the compiler's fast
weight load (FWL — disabled for fp32/fp32r), hiding the per-matmul LDWEIGHTS
that cost the fp32r version ~22ns/matmul, and input DMA bytes halve.
Accumulation stays fp32 in PSUM; stage C (relu * W3 + accumulate) runs in
fp32 on the DVE, so the only precision loss is fp16 operand rounding
(measured 5e-4 rel-to-max vs the fp32 reference, gate is 2e-2).

Per-core dataflow (per x-row i):
  stage A (ACT): h1T[h, j] = relu(hyT[h, j] + (hx_i[h] + b1[h]))     4x [128,512]
  stage B (PE):  x2[j, k]  = h1_i @ W2   (fp16 matmuls, psum accum)  16x mm
  stage C (DVE): s_i[j]    = sum_k relu(x2[j, k]) * W3[k]            4x scalar_tensor_tensor
with hxT = (x @ W1[:128]).T + b1 and hyT = (y @ W1[128:]).T computed once at
setup. b3 is applied on the host; b2 (nonzero only) via an extra K=1 matmul.
"""

import numpy as np

B = 512
DX = 128
H = 512
N_CORES = 8
ROWS = B // N_CORES  # 64 x-rows per core
HC = H // 128  # 4 chunks of the hidden dim

_BUILT = {}  # with_b2 -> bass.Bass


def _build(with_b2: bool):
    import concourse.mybir as mybir
    from concourse.bacc import Bacc
    from concourse.tile import TileContext

    F32 = mybir.dt.float32
    F16 = mybir.dt.float16
    Relu = mybir.ActivationFunctionType.Relu
    Alu = mybir.AluOpType

    # Bacc (not raw Bass): its compile pipeline splits multi-semaphore waits
    # into event-semaphore chains — TRN2 engine instructions accept only one
    # sync wait, which walrus otherwise rejects.
    nc = Bacc()
    xT_d = nc.declare_dram_parameter("xT", [DX, ROWS], F16, isOutput=False)
    yT_d = nc.declare_dram_parameter("yT", [DX, B], F16, isOutput=False)
    w1_d = nc.declare_dram_parameter("W1", [2 * DX, H], F16, isOutput=False)
    b1_d = nc.declare_dram_parameter("b1", [H], F32, isOutput=False)
    w2_d = nc.declare_dram_parameter("W2", [H, H], F16, isOutput=False)
    w3_d = nc.declare_dram_parameter("W3", [H, 1], F32, isOutput=False)
    if with_b2:
        b2_d = nc.declare_dram_parameter("b2", [H], F16, isOutput=False)
    out_d = nc.declare_dram_parameter("outT", [HC, 128, ROWS], F32, isOutput=True)

    with TileContext(nc) as tc:
        with (
            tc.tile_pool(name="consts", bufs=1) as cpool,
            tc.tile_pool(name="work", bufs=2) as wpool,
            tc.tile_pool(name="psum", bufs=8, space="PSUM") as ppool,
        ):
            # ---------------- input DMAs ----------------
            # Only the two HWDGE rings (sync/SP and scalar/ACT) — the gpsimd
            # SWDGE ring has a slow software drain in the epilogue. Within
            # each ring the earliest-needed tensor goes first.
            # sync ring: yT (gates hy), then all of W2 in one transfer.
            yT = cpool.tile([DX, B], F16, name="yT")
            nc.sync.dma_start(out=yT[:], in_=yT_d[:, :])
            w2sb = cpool.tile([128, HC, H], F16, name="w2sb")
            nc.sync.dma_start(
                out=w2sb[:], in_=w2_d[:, :].rearrange("(c p) k -> p c k", p=128)
            )
            # scalar ring: w1y (gates hy), w1x, xT, then the small tensors.
            w1y = cpool.tile([DX, H], F16, name="w1y")
            nc.scalar.dma_start(out=w1y[:], in_=w1_d[DX : 2 * DX, :])
            w1x = cpool.tile([DX, H], F16, name="w1x")
            nc.scalar.dma_start(out=w1x[:], in_=w1_d[0:DX, :])
            xT = cpool.tile([DX, ROWS], F16, name="xT")
            nc.scalar.dma_start(out=xT[:], in_=xT_d[:, :])
            b1sb = cpool.tile([128, HC], F32, name="b1sb")  # [p, hc]
            nc.scalar.dma_start(out=b1sb[:], in_=b1_d[:].rearrange("(c p) -> p c", p=128))
            w3row = cpool.tile([1, H], F32, name="w3row")
            nc.scalar.dma_start(out=w3row[:], in_=w3_d[:, :].flatten().unsqueeze(0))
            if with_b2:
                b2row = cpool.tile([1, H], F16, name="b2row")
                nc.scalar.dma_start(out=b2row[:], in_=b2_d[:].unsqueeze(0))
                ones1 = cpool.tile([1, 128], F16, name="ones1")
                nc.vector.memset(ones1[:], 1.0)

            # W3 broadcast to all 128 partitions via a K=1 ones-matmul
            # (gpsimd partition_broadcast would wake the SWDGE ring; a
            # stride-0 DMA would re-read the 2KB row 128 times from HBM).
            ones_col = cpool.tile([1, 128], F32, name="ones_col")
            nc.vector.memset(ones_col[:], 1.0)
            psb = ppool.tile([128, H], F32, name="psb", tag="ps")
            nc.tensor.matmul(psb[:], ones_col[:], w3row[:], start=True, stop=True)
            w3b = cpool.tile([128, H], F32, name="w3b")
            nc.vector.tensor_copy(out=w3b[:], in_=psb[:])

            # hyT[hc][h, j] = (y @ W1y).T stored fp16 (stage-A input)
            hyT = []
            for hc in range(HC):
                pshy = ppool.tile([128, B], F32, name="pshy", tag="ps")
                nc.tensor.matmul(
                    pshy[:], w1y[:, hc * 128 : (hc + 1) * 128], yT[:], start=True, stop=True
                )
                t = cpool.tile([128, B], F16, name=f"hyT_{hc}")
                nc.vector.tensor_copy(out=t[:], in_=pshy[:])
                hyT.append(t)

            # hxT[hc][h, i] = (x @ W1x).T + b1 kept fp32 (per-partition bias)
            hxT = []
            for hc in range(HC):
                pshx = ppool.tile([128, ROWS], F32, name="pshx", tag="ps")
                nc.tensor.matmul(
                    pshx[:], w1x[:, hc * 128 : (hc + 1) * 128], xT[:], start=True, stop=True
                )
                t = cpool.tile([128, ROWS], F32, name=f"hxT_{hc}")
                nc.vector.tensor_scalar_add(t[:], pshx[:], b1sb[:, hc : hc + 1])
                hxT.append(t)

            # scores accumulated transposed: scoresT[jc][j, i]
            scoresT = [cpool.tile([128, ROWS], F32, name=f"scT_{jc}") for jc in range(HC)]

            # ---------------- main loop over x rows ----------------
            for i in range(ROWS):
                h1T = []
                for hc in range(HC):
                    # ACT: relu(hyT + hx_i). All of stage A lives on ACT so the
                    # DVE has headroom for the stage-C fused reduce.
                    t = wpool.tile([128, B], F16, name="h1T", tag="h1T", bufs=12)
                    nc.scalar.activation(
                        t[:], hyT[hc][:], Relu, bias=hxT[hc][:, i : i + 1], scale=1.0
                    )
                    h1T.append(t)
                for jc in range(HC):
                    ps2 = ppool.tile([128, B], F32, name="ps2", tag="ps")
                    for hc in range(HC):
                        nc.tensor.matmul(
                            ps2[:],
                            h1T[hc][:, jc * 128 : (jc + 1) * 128],
                            w2sb[:, hc, :],
                            start=(hc == 0),
                            stop=(hc == HC - 1 and not with_b2),
                        )
                    if with_b2:
                        nc.tensor.matmul(
                            ps2[:], ones1[:], b2row[:], start=False, stop=True
                        )
                    # DVE: scr = relu(ps2) * W3_bcast; scoresT col = sum_k scr
                    scr = wpool.tile([128, B], F32, name="scr", tag="scr", bufs=6)
                    nc.vector.scalar_tensor_tensor(
                        out=scr[:],
                        in0=ps2[:],
                        scalar=0.0,
                        in1=w3b[:],
                        op0=Alu.max,
                        op1=Alu.mult,
                        accum_out=scoresT[jc][:, i : i + 1],
                    )

            # ---------------- store (host un-transposes) ----------------
            for jc in range(HC):
                nc.sync.dma_start(out=out_d[jc, :, :], in_=scoresT[jc][:])

    nc.finalize()  # runs the Bacc pass pipeline (wait splitting etc.)
    return nc


def _get_nc(with_b2: bool):
    if with_b2 not in _BUILT:
        _BUILT[with_b2] = _build(with_b2)
    return _BUILT[with_b2]


def _run(inputs: dict, trace: bool = False, **spmd_kwargs):
    """Shard, execute on 8 cores, gather. Returns (scores, BassKernelResults)."""
    from concourse.bass_utils import run_bass_kernel_spmd

    x = np.asarray(inputs["x"], dtype=np.float32)
    y = np.asarray(inputs["y"], dtype=np.float32)
    W1 = np.asarray(inputs["W1"], dtype=np.float32)
    b1 = np.ascontiguousarray(np.asarray(inputs["b1"], dtype=np.float32))
    W2 = np.asarray(inputs["W2"], dtype=np.float32)
    b2 = np.ascontiguousarray(np.asarray(inputs.get("b2", np.zeros(H)), dtype=np.float32))
    W3 = np.ascontiguousarray(np.asarray(inputs["W3"], dtype=np.float32).reshape(H, 1))
    b3 = np.asarray(inputs.get("b3", np.zeros(1)), dtype=np.float32)

    with_b2 = bool(np.any(b2))
    nc = _get_nc(with_b2)

    W1h = np.ascontiguousarray(W1.astype(np.float16))
    W2h = np.ascontiguousarray(W2.astype(np.float16))
    yTh = np.ascontiguousarray(y.T.astype(np.float16))
    xh = x.astype(np.float16)
    in_maps = []
    for c in range(N_CORES):
        m = {
            "xT": np.ascontiguousarray(xh[c * ROWS : (c + 1) * ROWS].T),
            "yT": yTh,
            "W1": W1h,
            "b1": b1,
            "W2": W2h,
            "W3": W3,
        }
        if with_b2:
            m["b2"] = np.ascontiguousarray(b2.astype(np.float16))
        in_maps.append(m)

    res = run_bass_kernel_spmd(
        nc, in_maps, core_ids=list(range(N_CORES)), trace=trace, **spmd_kwargs
    )
    # outT[jc, j, i] -> scores_slab[i, jc*128 + j]
    slabs = [
        np.transpose(r["outT"], (2, 0, 1)).reshape(ROWS, B) for r in res.results
    ]
    out = np.concatenate(slabs, axis=0)
    if b3.size and np.any(b3):
        out = out + b3.reshape(-1)[0]
    return np.ascontiguousarray(out.astype(np.float32)), res


def kernel(**inputs) -> np.ndarray:
    out, _ = _run(inputs)
    return out


# revision 4
# speedup vs baseline: 1.0585x; 1.0008x over previous
"""ConcatCritic pair-MLP kernel for 8 Trainium2 NeuronCores.

scores[i, j] = MLP(concat(x_i, y_j)) with
MLP = Linear(256,512) -> ReLU -> Linear(512,512) -> ReLU -> Linear(512,1).

Sharding: pure data parallelism over the B^2 pair grid, split along the
x (row) index — each core gets 64 rows of x plus all of y and the full
(small) weight set, and produces a [64, 512] slab of the output.

The host passes x and y pre-transposed (xT [128,64] slab, yT [128,512]) and
receives the output in transposed chunk layout [4][128 j][64 i]; both
transposes are trivial numpy work and remove every on-device transpose.

All matmul operands are fp16 (host-converted): the PE runs fp16 at the same
1 cycle/row as fp32r, but fp16 stationaries re-enable the compiler's fast
weight load (FWL — disabled for fp32/fp32r), hiding the per-matmul LDWEIGHTS
that cost the fp32r version ~22ns/matmul, and input DMA bytes halve.
Accumulation stays fp32 in PSUM; stage C (relu * W3 + accumulate) runs in
fp32 on the DVE, so the only precision loss is fp16 operand rounding
(measured 5e-4 rel-to-max vs the fp32 reference, gate is 2e-2).

Per-core dataflow (per x-row i):
  stage A (ACT): h1T[h, j] = relu(hyT[h, j] + (hx_i[h] + b1[h]))     4x [128,512]
  stage B (PE):  x2[j, k]  = h1_i @ W2   (fp16 matmuls, psum accum)  16x mm
  stage C (DVE): s_i[j]    = sum_k relu(x2[j, k]) * W3[k]            4x scalar_tensor_tensor
with hxT = (x @ W1[:128]).T + b1 and hyT = (y @ W1[128:]).T computed once at
setup. b3 is applied on the host; b2 (nonzero only) via an extra K=1 matmul.
"""

import numpy as np

B = 512
DX = 128
H = 512
N_CORES = 8
ROWS = B // N_CORES  # 64 x-rows per core
HC = H // 128  # 4 chunks of the hidden dim

_BUILT = {}  # with_b2 -> bass.Bass


def _build(with_b2: bool):
    import concourse.mybir as mybir
    from concourse.bacc import Bacc
    from concourse.tile import TileContext

    F32 = mybir.dt.float32
    F16 = mybir.dt.float16
    Relu = mybir.ActivationFunctionType.Relu
    Alu = mybir.AluOpType

    # Bacc (not raw Bass): its compile pipeline splits multi-semaphore waits
    # into event-semaphore chains — TRN2 engine instructions accept only one
    # sync wait, which walrus otherwise rejects.
    nc = Bacc()
    xT_d = nc.declare_dram_parameter("xT", [DX, ROWS], F16, isOutput=False)
    yT_d = nc.declare_dram_parameter("yT", [DX, B], F16, isOutput=False)
    w1_d = nc.declare_dram_parameter("W1", [2 * DX, H], F16, isOutput=False)
    b1_d = nc.declare_dram_parameter("b1", [H], F32, isOutput=False)
    w2_d = nc.declare_dram_parameter("W2", [H, H], F16, isOutput=False)
    w3_d = nc.declare_dram_parameter("W3", [H, 1], F32, isOutput=False)
    if with_b2:
        b2_d = nc.declare_dram_parameter("b2", [H], F16, isOutput=False)
    out_d = nc.declare_dram_parameter("outT", [HC, 128, ROWS], F32, isOutput=True)

    with TileContext(nc) as tc:
        with (
            tc.tile_pool(name="consts", bufs=1) as cpool,
            tc.tile_pool(name="work", bufs=2) as wpool,
            tc.tile_pool(name="psum", bufs=8, space="PSUM") as ppool,
        ):
            # ---------------- input DMAs ----------------
            # Only the two HWDGE rings (sync/SP and scalar/ACT) — the gpsimd
            # SWDGE ring has a slow software drain in the epilogue. Within
            # each ring the earliest-needed tensor goes first.
            # sync ring: yT (gates hy), then W2 chunks 0,2; scalar ring: w1y,
            # xT, W2 chunks 1,3, smalls. W2 is split per chunk so the first
            # layer-2 matmul (needs chunk 0 only) isn't gated on all 512KB.
            yT = cpool.tile([DX, B], F16, name="yT")
            nc.sync.dma_start(out=yT[:], in_=yT_d[:, :])
            w2sb = [cpool.tile([128, H], F16, name=f"w2_{hc}") for hc in range(HC)]
            w2r = w2_d[:, :].rearrange("(c p) k -> p c k", p=128)
            nc.sync.dma_start(out=w2sb[0][:], in_=w2r[:, 0, :])
            # scalar ring: w1y (gates hy), w1x, xT, then the small tensors.
            w1y = cpool.tile([DX, H], F16, name="w1y")
            nc.scalar.dma_start(out=w1y[:], in_=w1_d[DX : 2 * DX, :])
            w1x = cpool.tile([DX, H], F16, name="w1x")
            nc.scalar.dma_start(out=w1x[:], in_=w1_d[0:DX, :])
            xT = cpool.tile([DX, ROWS], F16, name="xT")
            nc.scalar.dma_start(out=xT[:], in_=xT_d[:, :])
            nc.scalar.dma_start(out=w2sb[1][:], in_=w2r[:, 1, :])
            nc.sync.dma_start(out=w2sb[2][:], in_=w2r[:, 2, :])
            nc.scalar.dma_start(out=w2sb[3][:], in_=w2r[:, 3, :])
            b1sb = cpool.tile([128, HC], F32, name="b1sb")  # [p, hc]
            nc.scalar.dma_start(out=b1sb[:], in_=b1_d[:].rearrange("(c p) -> p c", p=128))
            w3row = cpool.tile([1, H], F32, name="w3row")
            nc.scalar.dma_start(out=w3row[:], in_=w3_d[:, :].flatten().unsqueeze(0))
            if with_b2:
                b2row = cpool.tile([1, H], F16, name="b2row")
                nc.scalar.dma_start(out=b2row[:], in_=b2_d[:].unsqueeze(0))
                ones1 = cpool.tile([1, 128], F16, name="ones1")
                nc.vector.memset(ones1[:], 1.0)

            # W3 broadcast to all 128 partitions via a K=1 ones-matmul
            # (gpsimd partition_broadcast would wake the SWDGE ring; a
            # stride-0 DMA would re-read the 2KB row 128 times from HBM).
            ones_col = cpool.tile([1, 128], F32, name="ones_col")
            nc.vector.memset(ones_col[:], 1.0)
            psb = ppool.tile([128, H], F32, name="psb", tag="ps")
            nc.tensor.matmul(psb[:], ones_col[:], w3row[:], start=True, stop=True)
            w3b = cpool.tile([128, H], F32, name="w3b")
            nc.vector.tensor_copy(out=w3b[:], in_=psb[:])

            # hyT[hc][h, j] = (y @ W1y).T stored fp16 (stage-A input)
            hyT = []
            for hc in range(HC):
                pshy = ppool.tile([128, B], F32, name="pshy", tag="ps")
                nc.tensor.matmul(
                    pshy[:], w1y[:, hc * 128 : (hc + 1) * 128], yT[:], start=True, stop=True
                )
                t = cpool.tile([128, B], F16, name=f"hyT_{hc}")
                nc.vector.tensor_copy(out=t[:], in_=pshy[:])
                hyT.append(t)

            # hxT[hc][h, i] = (x @ W1x).T + b1 kept fp32 (per-partition bias)
            hxT = []
            for hc in range(HC):
                pshx = ppool.tile([128, ROWS], F32, name="pshx", tag="ps")
                nc.tensor.matmul(
                    pshx[:], w1x[:, hc * 128 : (hc + 1) * 128], xT[:], start=True, stop=True
                )
                t = cpool.tile([128, ROWS], F32, name=f"hxT_{hc}")
                nc.vector.tensor_scalar_add(t[:], pshx[:], b1sb[:, hc : hc + 1])
                hxT.append(t)

            # scores accumulated transposed: scoresT[jc][j, i]
            scoresT = [cpool.tile([128, ROWS], F32, name=f"scT_{jc}") for jc in range(HC)]

            # ---------------- main loop over x rows ----------------
            for i in range(ROWS):
                h1T = []
                for hc in range(HC):
                    # ACT: relu(hyT + hx_i). All of stage A lives on ACT so the
                    # DVE has headroom for the stage-C fused reduce.
                    t = wpool.tile([128, B], F16, name="h1T", tag="h1T", bufs=12)
                    nc.scalar.activation(
                        t[:], hyT[hc][:], Relu, bias=hxT[hc][:, i : i + 1], scale=1.0
                    )
                    h1T.append(t)
                for jc in range(HC):
                    ps2 = ppool.tile([128, B], F32, name="ps2", tag="ps")
                    for hc in range(HC):
                        nc.tensor.matmul(
                            ps2[:],
                            h1T[hc][:, jc * 128 : (jc + 1) * 128],
                            w2sb[hc][:],
                            start=(hc == 0),
                            stop=(hc == HC - 1 and not with_b2),
                        )
                    if with_b2:
                        nc.tensor.matmul(
                            ps2[:], ones1[:], b2row[:], start=False, stop=True
                        )
                    # DVE: scr = relu(ps2) * W3_bcast; scoresT col = sum_k scr
                    scr = wpool.tile([128, B], F32, name="scr", tag="scr", bufs=6)
                    nc.vector.scalar_tensor_tensor(
                        out=scr[:],
                        in0=ps2[:],
                        scalar=0.0,
                        in1=w3b[:],
                        op0=Alu.max,
                        op1=Alu.mult,
                        accum_out=scoresT[jc][:, i : i + 1],
                    )

            # ---------------- store (host un-transposes) ----------------
            for jc in range(HC):
                nc.sync.dma_start(out=out_d[jc, :, :], in_=scoresT[jc][:])

    nc.finalize()  # runs the Bacc pass pipeline (wait splitting etc.)
    return nc


def _get_nc(with_b2: bool):
    if with_b2 not in _BUILT:
        _BUILT[with_b2] = _build(with_b2)
    return _BUILT[with_b2]


def _run(inputs: dict, trace: bool = False, **spmd_kwargs):
    """Shard, execute on 8 cores, gather. Returns (scores, BassKernelResults)."""
    from concourse.bass_utils import run_bass_kernel_spmd

    x = np.asarray(inputs["x"], dtype=np.float32)
    y = np.asarray(inputs["y"], dtype=np.float32)
    W1 = np.asarray(inputs["W1"], dtype=np.float32)
    b1 = np.ascontiguousarray(np.asarray(inputs["b1"], dtype=np.float32))
    W2 = np.asarray(inputs["W2"], dtype=np.float32)
    b2 = np.ascontiguousarray(np.asarray(inputs.get("b2", np.zeros(H)), dtype=np.float32))
    W3 = np.ascontiguousarray(np.asarray(inputs["W3"], dtype=np.float32).reshape(H, 1))
    b3 = np.asarray(inputs.get("b3", np.zeros(1)), dtype=np.float32)

    with_b2 = bool(np.any(b2))
    nc = _get_nc(with_b2)

    W1h = np.ascontiguousarray(W1.astype(np.float16))
    W2h = np.ascontiguousarray(W2.astype(np.float16))
    yTh = np.ascontiguousarray(y.T.astype(np.float16))
    xh = x.astype(np.float16)
    in_maps = []
    for c in range(N_CORES):
        m = {
            "xT": np.ascontiguousarray(xh[c * ROWS : (c + 1) * ROWS].T),
            "yT": yTh,
            "W1": W1h,
            "b1": b1,
            "W2": W2h,
            "W3": W3,
        }
        if with_b2:
            m["b2"] = np.ascontiguousarray(b2.astype(np.float16))
        in_maps.append(m)

    res = run_bass_kernel_spmd(
        nc, in_maps, core_ids=list(range(N_CORES)), trace=trace, **spmd_kwargs
    )
    # outT[jc, j, i] -> scores_slab[i, jc*128 + j]
    slabs = [
        np.transpose(r["outT"], (2, 0, 1)).reshape(ROWS, B) for r in res.results
    ]
    out = np.concatenate(slabs, axis=0)
    if b3.size and np.any(b3):
        out = out + b3.reshape(-1)[0]
    return np.ascontiguousarray(out.astype(np.float32)), res


def kernel(**inputs) -> np.ndarray:
    out, _ = _run(inputs)
    return out


# revision 5
# speedup vs baseline: 1.0806x; 1.0209x over previous
"""ConcatCritic pair-MLP kernel for 8 Trainium2 NeuronCores.

scores[i, j] = MLP(concat(x_i, y_j)) with
MLP = Linear(256,512) -> ReLU -> Linear(512,512) -> ReLU -> Linear(512,1).

Sharding: pure data parallelism over the B^2 pair grid, split along the
x (row) index — each core gets 64 rows of x plus all of y and the full
(small) weight set, and produces a [64, 512] slab of the output.

The host passes x and y pre-transposed (xT [128,64] slab, yT [128,512]) and
receives the output in transposed chunk layout [4][128 j][64 i]; both
transposes are trivial numpy work and remove every on-device transpose.

All matmul operands are fp16 (host-converted): the PE runs fp16 at the same
1 cycle/row as fp32r, but fp16 stationaries re-enable the compiler's fast
weight load (FWL — disabled for fp32/fp32r), hiding the per-matmul LDWEIGHTS
that cost the fp32r version ~22ns/matmul, and input DMA bytes halve.
Accumulation stays fp32 in PSUM; stage C (relu * W3 + accumulate) runs in
fp32 on the DVE, so the only precision loss is fp16 operand rounding
(measured 5e-4 rel-to-max vs the fp32 reference, gate is 2e-2).

Per-core dataflow (per x-row i):
  stage A (ACT): h1T[h, j] = relu(hyT[h, j] + (hx_i[h] + b1[h]))     4x [128,512]
  stage B (PE):  x2[j, k]  = h1_i @ W2   (fp16 matmuls, psum accum)  16x mm
  stage C (DVE): s_i[j]    = sum_k relu(x2[j, k]) * W3[k]            4x scalar_tensor_tensor
with hxT = (x @ W1[:128]).T + b1 and hyT = (y @ W1[128:]).T computed once at
setup. b3 is applied on the host; b2 (nonzero only) via an extra K=1 matmul.
"""

import numpy as np

B = 512
DX = 128
H = 512
N_CORES = 8
ROWS = B // N_CORES  # 64 x-rows per core
HC = H // 128  # 4 chunks of the hidden dim

_BUILT = {}  # with_b2 -> bass.Bass


def _build(with_b2: bool):
    import concourse.mybir as mybir
    from concourse.bacc import Bacc
    from concourse.tile import TileContext

    F32 = mybir.dt.float32
    F16 = mybir.dt.float16
    Relu = mybir.ActivationFunctionType.Relu
    Alu = mybir.AluOpType

    # Bacc (not raw Bass): its compile pipeline splits multi-semaphore waits
    # into event-semaphore chains — TRN2 engine instructions accept only one
    # sync wait, which walrus otherwise rejects.
    nc = Bacc()
    xT_d = nc.declare_dram_parameter("xT", [DX, ROWS], F16, isOutput=False)
    yT_d = nc.declare_dram_parameter("yT", [DX, B], F16, isOutput=False)
    w1_d = nc.declare_dram_parameter("W1", [2 * DX, H], F16, isOutput=False)
    b1_d = nc.declare_dram_parameter("b1", [H], F32, isOutput=False)
    w2_d = nc.declare_dram_parameter("W2", [H, H], F16, isOutput=False)
    w3_d = nc.declare_dram_parameter("W3", [H, 1], F32, isOutput=False)
    if with_b2:
        b2_d = nc.declare_dram_parameter("b2", [H], F16, isOutput=False)
    out_d = nc.declare_dram_parameter("outT", [HC, 128, ROWS], F32, isOutput=True)

    with TileContext(nc) as tc:
        with (
            tc.tile_pool(name="consts", bufs=1) as cpool,
            tc.tile_pool(name="work", bufs=2) as wpool,
            tc.tile_pool(name="psum", bufs=8, space="PSUM") as ppool,
        ):
            # ---------------- input DMAs ----------------
            # Only the two HWDGE rings (sync/SP and scalar/ACT) — the gpsimd
            # SWDGE ring has a slow software drain in the epilogue. Within
            # each ring the earliest-needed tensor goes first.
            # sync ring: yT (gates hy), then W2 chunks 0,2; scalar ring: w1y,
            # xT, W2 chunks 1,3, smalls. W2 is split per chunk so the first
            # layer-2 matmul (needs chunk 0 only) isn't gated on all 512KB.
            yT = cpool.tile([DX, B], F16, name="yT")
            nc.sync.dma_start(out=yT[:], in_=yT_d[:, :])
            w2sb = [cpool.tile([128, H], F16, name=f"w2_{hc}") for hc in range(HC)]
            w2r = w2_d[:, :].rearrange("(c p) k -> p c k", p=128)
            nc.sync.dma_start(out=w2sb[0][:], in_=w2r[:, 0, :])
            # scalar ring: w1y (gates hy), w1x, xT, then the small tensors.
            w1y = cpool.tile([DX, H], F16, name="w1y")
            nc.scalar.dma_start(out=w1y[:], in_=w1_d[DX : 2 * DX, :])
            w1x = cpool.tile([DX, H], F16, name="w1x")
            nc.scalar.dma_start(out=w1x[:], in_=w1_d[0:DX, :])
            xT = cpool.tile([DX, ROWS], F16, name="xT")
            nc.scalar.dma_start(out=xT[:], in_=xT_d[:, :])
            b1sb = cpool.tile([128, HC], F32, name="b1sb")  # [p, hc]
            nc.scalar.dma_start(out=b1sb[:], in_=b1_d[:].rearrange("(c p) -> p c", p=128))
            w3row = cpool.tile([1, H], F32, name="w3row")
            nc.scalar.dma_start(out=w3row[:], in_=w3_d[:, :].flatten().unsqueeze(0))
            nc.scalar.dma_start(out=w2sb[1][:], in_=w2r[:, 1, :])
            nc.sync.dma_start(out=w2sb[2][:], in_=w2r[:, 2, :])
            nc.scalar.dma_start(out=w2sb[3][:], in_=w2r[:, 3, :])
            if with_b2:
                b2row = cpool.tile([1, H], F16, name="b2row")
                nc.scalar.dma_start(out=b2row[:], in_=b2_d[:].unsqueeze(0))
                ones1 = cpool.tile([1, 128], F16, name="ones1")
                nc.vector.memset(ones1[:], 1.0)

            # hyT[hc][h, j] = (y @ W1y).T stored fp16 (stage-A input)
            hyT = []
            for hc in range(HC):
                pshy = ppool.tile([128, B], F32, name="pshy", tag="ps")
                nc.tensor.matmul(
                    pshy[:], w1y[:, hc * 128 : (hc + 1) * 128], yT[:], start=True, stop=True
                )
                t = cpool.tile([128, B], F16, name=f"hyT_{hc}")
                nc.vector.tensor_copy(out=t[:], in_=pshy[:])
                hyT.append(t)

            # hxT[hc][h, i] = (x @ W1x).T + b1 kept fp32 (per-partition bias)
            hxT = []
            for hc in range(HC):
                pshx = ppool.tile([128, ROWS], F32, name="pshx", tag="ps")
                nc.tensor.matmul(
                    pshx[:], w1x[:, hc * 128 : (hc + 1) * 128], xT[:], start=True, stop=True
                )
                t = cpool.tile([128, ROWS], F32, name=f"hxT_{hc}")
                nc.vector.tensor_scalar_add(t[:], pshx[:], b1sb[:, hc : hc + 1])
                hxT.append(t)

            # W3 broadcast to all 128 partitions via a K=1 ones-matmul
            # (gpsimd partition_broadcast would wake the SWDGE ring; a
            # stride-0 DMA would re-read the 2KB row 128 times from HBM).
            ones_col = cpool.tile([1, 128], F32, name="ones_col")
            nc.vector.memset(ones_col[:], 1.0)
            psb = ppool.tile([128, H], F32, name="psb", tag="ps")
            nc.tensor.matmul(psb[:], ones_col[:], w3row[:], start=True, stop=True)
            w3b = cpool.tile([128, H], F32, name="w3b")
            nc.vector.tensor_copy(out=w3b[:], in_=psb[:])

            # scores accumulated transposed: scoresT[jc][j, i]
            scoresT = [cpool.tile([128, ROWS], F32, name=f"scT_{jc}") for jc in range(HC)]

            # ---------------- main loop over x rows ----------------
            for i in range(ROWS):
                h1T = []
                for hc in range(HC):
                    # ACT: relu(hyT + hx_i). All of stage A lives on ACT so the
                    # DVE has headroom for the stage-C fused reduce.
                    t = wpool.tile([128, B], F16, name="h1T", tag="h1T", bufs=12)
                    nc.scalar.activation(
                        t[:], hyT[hc][:], Relu, bias=hxT[hc][:, i : i + 1], scale=1.0
                    )
                    h1T.append(t)
                for jc in range(HC):
                    ps2 = ppool.tile([128, B], F32, name="ps2", tag="ps")
                    for hc in range(HC):
                        nc.tensor.matmul(
                            ps2[:],
                            h1T[hc][:, jc * 128 : (jc + 1) * 128],
                            w2sb[hc][:],
                            start=(hc == 0),
                            stop=(hc == HC - 1 and not with_b2),
                        )
                    if with_b2:
                        nc.tensor.matmul(
                            ps2[:], ones1[:], b2row[:], start=False, stop=True
                        )
                    # DVE: scr = relu(ps2) * W3_bcast; scoresT col = sum_k scr
                    scr = wpool.tile([128, B], F32, name="scr", tag="scr", bufs=6)
                    nc.vector.scalar_tensor_tensor(
                        out=scr[:],
                        in0=ps2[:],
                        scalar=0.0,
                        in1=w3b[:],
                        op0=Alu.max,
                        op1=Alu.mult,
                        accum_out=scoresT[jc][:, i : i + 1],
                    )

            # ---------------- store (host un-transposes) ----------------
            for jc in range(HC):
                eng = nc.sync if jc % 2 == 0 else nc.scalar
                eng.dma_start(out=out_d[jc, :, :], in_=scoresT[jc][:])

    nc.finalize()  # runs the Bacc pass pipeline (wait splitting etc.)
    return nc


def _get_nc(with_b2: bool):
    if with_b2 not in _BUILT:
        _BUILT[with_b2] = _build(with_b2)
    return _BUILT[with_b2]


def _run(inputs: dict, trace: bool = False, **spmd_kwargs):
    """Shard, execute on 8 cores, gather. Returns (scores, BassKernelResults)."""
    from concourse.bass_utils import run_bass_kernel_spmd

    x = np.asarray(inputs["x"], dtype=np.float32)
    y = np.asarray(inputs["y"], dtype=np.float32)
    W1 = np.asarray(inputs["W1"], dtype=np.float32)
    b1 = np.ascontiguousarray(np.asarray(inputs["b1"], dtype=np.float32))
    W2 = np.asarray(inputs["W2"], dtype=np.float32)
    b2 = np.ascontiguousarray(np.asarray(inputs.get("b2", np.zeros(H)), dtype=np.float32))
    W3 = np.ascontiguousarray(np.asarray(inputs["W3"], dtype=np.float32).reshape(H, 1))
    b3 = np.asarray(inputs.get("b3", np.zeros(1)), dtype=np.float32)

    with_b2 = bool(np.any(b2))
    nc = _get_nc(with_b2)

    W1h = np.ascontiguousarray(W1.astype(np.float16))
    W2h = np.ascontiguousarray(W2.astype(np.float16))
    yTh = np.ascontiguousarray(y.T.astype(np.float16))
    xh = x.astype(np.float16)
    in_maps = []
    for c in range(N_CORES):
        m = {
            "xT": np.ascontiguousarray(xh[c * ROWS : (c + 1) * ROWS].T),
            "yT": yTh,
            "W1": W1h,
            "b1": b1,
            "W2": W2h,
            "W3": W3,
        }
        if with_b2:
            m["b2"] = np.ascontiguousarray(b2.astype(np.float16))
        in_maps.append(m)

    res = run_bass_kernel_spmd(
        nc, in_maps, core_ids=list(range(N_CORES)), trace=trace, **spmd_kwargs
    )
    # outT[jc, j, i] -> scores_slab[i, jc*128 + j]
    slabs = [
        np.transpose(r["outT"], (2, 0, 1)).reshape(ROWS, B) for r in res.results
    ]
    out = np.concatenate(slabs, axis=0)
    if b3.size and np.any(b3):
        out = out + b3.reshape(-1)[0]
    return np.ascontiguousarray(out.astype(np.float32)), res


def kernel(**inputs) -> np.ndarray:
    out, _ = _run(inputs)
    return out
